# revision 1
# baseline (speedup 1.0000x reference)
"""GATNet Trainium kernel: host preprocessing + Bass program builder.

Design: 8-way dst-shard of nodes. Per layer:
  node phase (per-shard): [h|es|ed] = xT_shard.T @ [W | W@a_s | W@a_d]  (PE, bf16)
  exchange: AllGather of node table T=[h|es] (f32) -> every core has all rows
  aggregation (per-shard dst windows of 128):
    - indirect-DMA gather of T rows for each edge (128 edges/partition-tile)
    - per-edge w = exp(leakyrelu(es_src + ed_dst)) on ACT; ed via small indirect gather
    - V = [w*h | w] (DVE, bf16); segment-sum via matmuls with baked one-hot S (PE)
    - epilogue: out/s + bias, relu -> xT for next layer via PE transpose
  pool phase: baked node->graph one-hot matmuls + AllReduce + tiny MLP.
"""
import sys
sys.path.insert(0, "/opt/trn_rl_repo")
sys.path.insert(0, "/opt/trn_rl_repo/concourse")
import numpy as np
import ml_dtypes
from concourse import bass, bacc, mybir
import concourse.tile as tile

BF16 = mybir.dt.bfloat16
F32 = mybir.dt.float32
I32 = mybir.dt.int32
AF = mybir.ActivationFunctionType

N = 50000
E = 400000
G = 256
H = 2
C = 100
HC = 200
F_IN = 336
SLOPE = 0.2
NCORE = 8
SHARD = N // NCORE            # 6250
P = 128
SHARD_PAD = 6272              # 49*128
NW = 49                       # dst windows per core (128 dsts each; last 106)
NT = 49                       # node tiles per core
TCOLS = 202                   # T table: [h(200) | es(2)]
NFULL_PAD = NCORE * SHARD_PAD # T_full rows


def prep(x, edge_index, batch, Ws, asrcs, adsts, bcs, lws, lbs):
    """Host-side preprocessing. Returns (per_core_inputs, shared_inputs, meta)."""
    bf = np.float32
    src_all = np.concatenate([edge_index[0], np.arange(N, dtype=np.int64)])
    dst_all = np.concatenate([edge_index[1], np.arange(N, dtype=np.int64)])
    order = np.argsort(dst_all, kind="stable")
    s_sorted = src_all[order].astype(np.int64)
    d_sorted = dst_all[order].astype(np.int64)
    ET = len(s_sorted)

    # global T_full row index for each src (T_full = concat of padded shards)
    s_row = (s_sorted // SHARD) * SHARD_PAD + (s_sorted % SHARD)

    # per (core, window) edge ranges
    win_edges = []   # (core, w) -> (lo, hi) into sorted arrays
    smax = 0
    for k in range(NCORE):
        for w in range(NW):
            d_lo = k * SHARD + w * P
            d_hi = min(k * SHARD + min((w + 1) * P, SHARD), N)
            lo = np.searchsorted(d_sorted, d_lo, side="left")
            hi = np.searchsorted(d_sorted, d_hi, side="left")
            win_edges.append((lo, hi))
            smax = max(smax, -(-(hi - lo) // P))
    SMAX = smax
    SLOTS = SMAX * P

    per_core = []
    for k in range(NCORE):
        gidx = np.zeros((NW, P, SMAX), np.int32)
        sblob = np.zeros((NW, P, SMAX * P), bf)
        stblob = np.zeros((NW, P, SMAX * P), bf)
        for w in range(NW):
            lo, hi = win_edges[k * NW + w]
            ne = hi - lo
            srcs = np.zeros(SLOTS, np.int64)
            dloc = np.zeros(SLOTS, np.int64)
            srcs[:ne] = s_row[lo:hi]
            srcs[ne:] = s_row[lo] if ne > 0 else 0
            dloc[:ne] = d_sorted[lo:hi] - k * SHARD   # shard-local dst
            dloc[ne:] = 0
            sl = np.arange(SLOTS)
            gidx[w, sl % P, sl // P] = srcs            # slot s -> (p=s%P, j=s//P)
            S = np.zeros((SLOTS, P), np.float32)
            m = (dloc[:ne] - w * P).astype(np.int64)
            S[np.arange(ne), m] = 1.0
            # S blob: [slot, m] -> [p, j, m] with slot = j*P + p
            sblob[w] = (
                S.reshape(SMAX, P, P).transpose(1, 0, 2).reshape(P, SMAX * P).astype(bf)
            )
            # S_T blob: [m, j, p] -> lhsT tiles [128 dst, 128 slot] per j
            stblob[w] = (
                S.reshape(SMAX, P, P).transpose(2, 0, 1).reshape(P, SMAX * P).astype(bf)
            )
        # xT0 shard: [384, 6272]
        xT0 = np.zeros((384, SHARD_PAD), bf)
        xT0[:F_IN, :SHARD] = x[k * SHARD:(k + 1) * SHARD].T.astype(bf)
        # pool mask [49, 128, 256]
        pmask = np.zeros((NT, P, G), bf)
        bsh = batch[k * SHARD:(k + 1) * SHARD]
        for t in range(NT):
            r0 = t * P
            r1 = min(r0 + P, SHARD)
            if r1 > r0:
                pmask[t, np.arange(r1 - r0), bsh[r0:r1]] = 1.0
        per_core.append(dict(xT0=xT0, gidx=gidx, sblob=sblob, stblob=stblob, pmask=pmask))

    # shared weights
    waug = np.zeros((11, P, 204), bf)
    biasrep = np.zeros((5, P, TCOLS), np.float32)
    ci = 0
    for li in range(5):
        W = Ws[li].astype(np.float32)          # [F, 200]
        a_s, a_d = asrcs[li].astype(np.float32), adsts[li].astype(np.float32)  # [H, C]
        was = np.zeros((W.shape[0], 2), np.float32)
        wad = np.zeros((W.shape[0], 2), np.float32)
        for h in range(H):
            was[:, h] = W[:, h * C:(h + 1) * C] @ a_s[h]
            wad[:, h] = W[:, h * C:(h + 1) * C] @ a_d[h]
        aug = np.concatenate([W, was, wad], axis=1)  # [F, 204]
        nch = 3 if li == 0 else 2
        for c in range(nch):
            rows = aug[c * P:(c + 1) * P]
            waug[ci, :rows.shape[0]] = rows.astype(bf)
            ci += 1
        b = bcs[li].astype(np.float32)
        biasrep[li, :, 0:200] = b
    assert ci == 11

    mlw1 = lws[0].astype(bf)            # [200, 100]
    mlw2 = lws[1].astype(bf)            # [100, 100]
    mlw3 = lws[2].astype(bf)            # [100, 29]
    mlpb = np.zeros((3, P, 1), np.float32)
    mlpb[0, :100, 0] = lbs[0]
    mlpb[1, :100, 0] = lbs[1]
    mlpb[2, :29, 0] = lbs[2]
    ident = np.eye(P, dtype=bf)

    shared = dict(waug=waug, biasrep=biasrep, mlw1=mlw1, mlw2=mlw2, mlw3=mlw3,
                  mlpb=mlpb, ident=ident)
    meta = dict(SMAX=SMAX)
    return per_core, shared, meta


def build_nc(SMAX, dbg=False):
    nc = bacc.Bacc("TRN2", target_bir_lowering=False)
    SLOTS = SMAX * P

    # inputs
    xT0 = nc.declare_dram_parameter("xT0", [384, SHARD_PAD], F32, isOutput=False)
    gidx = nc.declare_dram_parameter("gidx", [NW, P, SMAX], I32, isOutput=False)
    stblob = nc.declare_dram_parameter("stblob", [NW, P, SLOTS], F32, isOutput=False)
    sblob = nc.declare_dram_parameter("sblob", [NW, P, SLOTS], F32, isOutput=False)
    pmask = nc.declare_dram_parameter("pmask", [NT, P, G], F32, isOutput=False)
    waug = nc.declare_dram_parameter("waug", [11, P, 204], F32, isOutput=False)
    biasrep = nc.declare_dram_parameter("biasrep", [5, P, TCOLS], F32, isOutput=False)
    mlw1 = nc.declare_dram_parameter("mlw1", [200, 100], F32, isOutput=False)
    mlw2 = nc.declare_dram_parameter("mlw2", [100, 100], F32, isOutput=False)
    mlw3 = nc.declare_dram_parameter("mlw3", [100, 29], F32, isOutput=False)
    mlpb = nc.declare_dram_parameter("mlpb", [3, P, 1], F32, isOutput=False)
    ident_in = nc.declare_dram_parameter("ident", [P, P], F32, isOutput=False)
    out = nc.declare_dram_parameter("out", [29, G], F32, isOutput=True)
    if dbg:
        dbg_T = nc.declare_dram_parameter("dbg_T", [SHARD_PAD, TCOLS], F32, isOutput=True)
        dbg_ed = nc.declare_dram_parameter("dbg_ed", [SHARD_PAD, 2], F32, isOutput=True)
        dbg_x1 = nc.declare_dram_parameter("dbg_x1", [256, SHARD_PAD], F32, isOutput=True)
        dbg_x6 = nc.declare_dram_parameter("dbg_x6", [SHARD_PAD, 201], F32, isOutput=True)
        dbg_pool = nc.declare_dram_parameter("dbg_pool", [201, G], F32, isOutput=True)
        dbg_G = nc.declare_dram_parameter("dbg_G", [P, SMAX * TCOLS], F32, isOutput=True)
        dbg_w = nc.declare_dram_parameter("dbg_w", [P, SMAX * 2], F32, isOutput=True)
        dbg_ap = nc.declare_dram_parameter("dbg_ap", [P, TCOLS], F32, isOutput=True)
        dbg_stb = nc.declare_dram_parameter("dbg_stb", [P, TCOLS], F32, isOutput=True)
        dbg_V = nc.declare_dram_parameter("dbg_V", [P, SMAX * TCOLS], F32, isOutput=True)

    # internal dram
    Tshard = nc.dram_tensor("Tshard", [SHARD_PAD, TCOLS], F32)
    Tfull = nc.dram_tensor("Tfull", [NFULL_PAD, TCOLS], F32, addr_space="Shared")
    ed_tab = nc.dram_tensor("ed_tab", [SHARD_PAD, 2], F32)
    xTa = nc.dram_tensor("xTa", [256, SHARD_PAD], F32)
    xTb = nc.dram_tensor("xTb", [256, SHARD_PAD], F32)
    x6 = nc.dram_tensor("x6", [SHARD_PAD, 201], F32)
    cc2_in = nc.dram_tensor("cc2_in", [201, G], F32)
    cc2_out = nc.dram_tensor("cc2_out", [201, G], F32, addr_space="Shared")
    rg = [list(range(NCORE))]

    with tile.TileContext(nc) as tc:
        with tc.tile_pool(name="const", bufs=1) as cpool:
            # resident constants
            wtiles = []
            for i in range(11):
                wt = cpool.tile([P, 204], F32, tag=f"waug{i}")
                nc.sync.dma_start(out=wt[:], in_=waug[i])
                wtiles.append(wt)
            ident = cpool.tile([P, P], F32, tag="ident")
            nc.sync.dma_start(out=ident[:], in_=ident_in[:])
            zpad = cpool.tile([56, SHARD_PAD // 4], F32, tag="zpad")
            nc.vector.memset(zpad[:], 0.0)
            onescol = cpool.tile([P, 1], F32, tag="onescol")
            nc.vector.memset(onescol[:], 1.0)
            ones1f = cpool.tile([1, P], F32, tag="ones1f")
            nc.vector.memset(ones1f[:], 1.0)

            # zero the pad rows (200:256) of xT ping-pong buffers
            for xb in (xTa, xTb):
                for q in range(4):
                    nc.sync.dma_start(
                        out=xb[200:256, q * (SHARD_PAD // 4):(q + 1) * (SHARD_PAD // 4)],
                        in_=zpad[:],
                    )

            with tc.tile_pool(name="sb", bufs=2) as pool, \
                 tc.tile_pool(name="sb3", bufs=3) as pool3:
              with tc.tile_pool(name="ps", bufs=2, space="PSUM") as pspool:

                def node_phase(li, src_dram, nch, ci0):
                    """h|es|ed for own shard -> Tshard + ed_tab."""
                    for t in range(NT):
                        r0 = t * P if t < NT - 1 else SHARD - P
                        npsum = pspool.tile([P, 204], F32, tag="npsum")
                        for c in range(nch):
                            lt = pool3.tile([P, P], F32, tag="nlhsT")
                            nc.sync.dma_start(
                                out=lt[:],
                                in_=src_dram[c * P:(c + 1) * P, r0:r0 + P])
                            nc.tensor.matmul(
                                out=npsum[:], lhsT=lt[:], rhs=wtiles[ci0 + c][:],
                                start=(c == 0), stop=(c == nch - 1))
                        ts = pool.tile([P, 204], F32, tag="tstage")
                        nc.vector.tensor_copy(out=ts[:], in_=npsum[:])
                        nc.sync.dma_start(out=Tshard[r0:r0 + P, :], in_=ts[:, 0:TCOLS])
                        nc.sync.dma_start(out=ed_tab[r0:r0 + P, :], in_=ts[:, TCOLS:204])

                def agg_phase(li, xT_next):
                    last = li == 4
                    brt = pool.tile([P, TCOLS], F32, tag="biasrep")
                    nc.sync.dma_start(out=brt[:], in_=biasrep[li])
                    for w in range(NW):
                        git = pool3.tile([P, SMAX], I32, tag="git")
                        nc.sync.dma_start(out=git[:], in_=gidx[w])
                        Gt = pool.tile([P, SMAX, TCOLS], F32, tag="Gt")
                        for j in range(SMAX):
                            nc.gpsimd.indirect_dma_start(
                                out=Gt[:, j, :], out_offset=None, in_=Tfull[:],
                                in_offset=bass.IndirectOffsetOnAxis(ap=git[:, j:j + 1], axis=0))
                        st = pool.tile([P, SLOTS], F32, tag="st")
                        nc.sync.dma_start(out=st[:], in_=sblob[w])
                        stt = pool.tile([P, SLOTS], F32, tag="stt")
                        nc.sync.dma_start(out=stt[:], in_=stblob[w])
                        edw = pool.tile([P, 2], F32, tag="edw")
                        nc.sync.dma_start(out=edw[:], in_=ed_tab[w * P:(w + 1) * P, :])
                        edp = pspool.tile([P, 2 * SMAX], F32, tag="edp")
                        for j in range(SMAX):
                            nc.tensor.matmul(
                                out=edp[:, 2 * j:2 * j + 2], lhsT=stt[:, j * P:(j + 1) * P],
                                rhs=edw[:], start=True, stop=True)
                        # scores
                        e1 = pool.tile([P, SMAX, 2], F32, tag="e1")
                        nc.vector.tensor_add(out=e1[:], in0=Gt[:, :, 200:202],
                                             in1=edp[:].rearrange("p (j c) -> p j c", c=2))
                        e2 = pool.tile([P, SMAX, 2], F32, tag="e2")
                        nc.vector.tensor_scalar_mul(out=e2[:], in0=e1[:], scalar1=SLOPE)
                        e3 = pool.tile([P, SMAX, 2], F32, tag="e3")
                        nc.vector.tensor_tensor(out=e3[:], in0=e1[:], in1=e2[:], op=mybir.AluOpType.max)
                        wv = pool.tile([P, SMAX, 2], F32, tag="wv")
                        nc.scalar.activation(out=wv[:], in_=e3[:], func=AF.Exp)
                        # V = [w*h | w] bf16
                        Vt = pool.tile([P, SMAX, TCOLS], F32, tag="Vt")
                        nc.vector.tensor_mul(
                            out=Vt[:, :, 0:200].rearrange("p j (h c) -> p j h c", h=H),
                            in0=Gt[:, :, 0:200].rearrange("p j (h c) -> p j h c", h=H),
                            in1=wv[:].unsqueeze(3).broadcast_to([P, SMAX, 2, C]))
                        nc.vector.tensor_copy(out=Vt[:, :, 200:202], in_=wv[:])
                        # segment sum
                        apsum = pspool.tile([P, TCOLS], F32, tag="apsum")
                        for j in range(SMAX):
                            nc.tensor.matmul(
                                out=apsum[:], lhsT=st[:, j * P:(j + 1) * P],
                                rhs=Vt[:, j, :], start=(j == 0), stop=(j == SMAX - 1))
                        if dbg and li == 0 and w == 0:
                            nc.sync.dma_start(out=dbg_G[:], in_=Gt[:].rearrange("p j c -> p (j c)"))
                            nc.sync.dma_start(out=dbg_w[:], in_=wv[:].rearrange("p j c -> p (j c)"))
                            nc.sync.dma_start(out=dbg_V[:], in_=Vt[:].rearrange("p j c -> p (j c)"))
                            apc = pool.tile([P, TCOLS], F32, tag="apc")
                            nc.vector.tensor_copy(out=apc[:], in_=apsum[:])
                            nc.sync.dma_start(out=dbg_ap[:], in_=apc[:])
                        # epilogue
                        sc = pool.tile([P, 2], F32, tag="sc")
                        nc.vector.tensor_scalar_add(out=sc[:], in0=apsum[:, 200:202], scalar1=1e-30)
                        rc = pool.tile([P, 2], F32, tag="rc")
                        nc.vector.reciprocal(out=rc[:], in_=sc[:])
                        stg = pool.tile([P, TCOLS], F32, tag="stg")
                        nc.vector.tensor_scalar_mul(out=stg[:, 0:100], in0=apsum[:, 0:100], scalar1=rc[:, 0:1])
                        nc.vector.tensor_scalar_mul(out=stg[:, 100:202], in0=apsum[:, 100:202], scalar1=rc[:, 1:2])
                        nc.vector.tensor_add(out=stg[:], in0=stg[:], in1=brt[:])
                        stb = pool.tile([P, TCOLS], F32, tag="stb")
                        nc.scalar.activation(out=stb[:], in_=stg[:], func=AF.Relu)
                        if dbg and li == 0 and w == 0:
                            nc.sync.dma_start(out=dbg_stb[:], in_=stb[:])
                        if not last:
                            tp1 = pspool.tile([P, P], F32, tag="tp")
                            nc.tensor.transpose(out=tp1[:], in_=stb[:, 0:128], identity=ident[:])
                            tp2 = pspool.tile([P, P], F32, tag="tp")
                            nc.tensor.transpose(out=tp2[:74, :], in_=stb[:, 128:202], identity=ident[:])
                            tr1 = pool.tile([P, P], F32, tag="tr1")
                            nc.vector.tensor_copy(out=tr1[:], in_=tp1[:])
                            tr2 = pool.tile([74, P], F32, tag="tr2")
                            nc.vector.tensor_copy(out=tr2[:], in_=tp2[:74, :])
                            c0 = w * P
                            nc.sync.dma_start(out=xT_next[0:128, c0:c0 + P], in_=tr1[:])
                            nc.sync.dma_start(out=xT_next[128:200, c0:c0 + P], in_=tr2[0:72, :])
                        else:
                            r0 = w * P
                            nc.sync.dma_start(out=x6[r0:r0 + P, 0:200], in_=stb[:, 0:200])
                            nc.sync.dma_start(out=x6[r0:r0 + P, 200:201], in_=onescol[:])

                # ---- 5 GAT layers ----
                srcs = [xT0, xTa, xTb, xTa, xTb]
                nxts = [xTa, xTb, xTa, xTb, None]
                for li in range(5):
                    nch = 3 if li == 0 else 2
                    ci0 = 0 if li == 0 else 3 + 2 * (li - 1)
                    node_phase(li, srcs[li], nch, ci0)
                    if dbg and li == 0:
                        nc.sync.dma_start(out=dbg_T[:], in_=Tshard[:])
                        nc.sync.dma_start(out=dbg_ed[:], in_=ed_tab[:])
                    tc.strict_bb_all_engine_barrier()
                    nc.gpsimd.collective_compute(
                        "AllGather", mybir.AluOpType.bypass, replica_groups=rg,
                        ins=[Tshard[:]], outs=[Tfull[:]])
                    tc.strict_bb_all_engine_barrier()
                    agg_phase(li, nxts[li])
                    if dbg and li == 0:
                        nc.sync.dma_start(out=dbg_x1[:], in_=xTa[:])
                    tc.strict_bb_all_engine_barrier()
                if dbg:
                    nc.sync.dma_start(out=dbg_x6[:], in_=x6[:])

              # ---- pool ----
              with tc.tile_pool(name="ps2", bufs=1, space="PSUM") as pspool:
                tc.strict_bb_all_engine_barrier()
                ppA = pspool.tile([P, G], F32, tag="ppA")
                ppB = pspool.tile([73, G], F32, tag="ppB")
                for t in range(NT):
                    x6t = pool3.tile([P, 201], F32, tag="x6t")
                    nc.sync.dma_start(out=x6t[:], in_=x6[t * P:(t + 1) * P, :])
                    pmt = pool3.tile([P, G], F32, tag="pmt")
                    nc.sync.dma_start(out=pmt[:], in_=pmask[t])
                    nc.tensor.matmul(out=ppA[:], lhsT=x6t[:, 0:128], rhs=pmt[:],
                                     start=(t == 0), stop=(t == NT - 1))
                    nc.tensor.matmul(out=ppB[:], lhsT=x6t[:, 128:201], rhs=pmt[:],
                                     start=(t == 0), stop=(t == NT - 1))
                pstA = pool.tile([P, G], F32, tag="pstA")
                nc.vector.tensor_copy(out=pstA[:], in_=ppA[:])
                pstB = pool.tile([73, G], F32, tag="pstB")
                nc.vector.tensor_copy(out=pstB[:], in_=ppB[:])
                nc.sync.dma_start(out=cc2_in[0:128, :], in_=pstA[:])
                nc.sync.dma_start(out=cc2_in[128:201, :], in_=pstB[:])
                tc.strict_bb_all_engine_barrier()
                nc.gpsimd.collective_compute(
                    "AllReduce", mybir.AluOpType.add, replica_groups=rg,
                    ins=[cc2_in[:]], outs=[cc2_out[:]])
                tc.strict_bb_all_engine_barrier()
                if dbg:
                    nc.sync.dma_start(out=dbg_pool[:], in_=cc2_out[:])
                plA = pool.tile([P, G], F32, tag="plA")
                nc.sync.dma_start(out=plA[:], in_=cc2_out[0:128, :])
                plB = pool.tile([73, G], F32, tag="plB")
                nc.sync.dma_start(out=plB[:], in_=cc2_out[128:201, :])
                cntr = pool.tile([1, G], F32, tag="cntr")
                nc.sync.dma_start(out=cntr[:], in_=cc2_out[200:201, :])
                cntm = pool.tile([1, G], F32, tag="cntm")
                nc.vector.tensor_scalar_max(out=cntm[:], in0=cntr[:], scalar1=1.0)
                rc2 = pool.tile([1, G], F32, tag="rc2")
                nc.vector.reciprocal(out=rc2[:], in_=cntm[:])
                Rb = pspool.tile([P, G], F32, tag="Rb")
                nc.tensor.matmul(out=Rb[:], lhsT=ones1f[:], rhs=rc2[:], start=True, stop=True)
                mA = pool.tile([P, G], F32, tag="mA")
                nc.vector.tensor_mul(out=mA[:], in0=plA[:], in1=Rb[:])
                mB = pool.tile([72, G], F32, tag="mB")
                nc.vector.tensor_mul(out=mB[:], in0=plB[0:72, :], in1=Rb[0:72, :])
                # MLP
                w1a = pool.tile([P, 100], F32, tag="w1a")
                nc.sync.dma_start(out=w1a[:], in_=mlw1[0:128, :])
                w1b = pool.tile([72, 100], F32, tag="w1b")
                nc.sync.dma_start(out=w1b[:], in_=mlw1[128:200, :])
                w2t = pool.tile([100, 100], F32, tag="w2t")
                nc.sync.dma_start(out=w2t[:], in_=mlw2[:])
                w3t = pool.tile([100, 29], F32, tag="w3t")
                nc.sync.dma_start(out=w3t[:], in_=mlw3[:])
                b1 = pool.tile([P, 1], F32, tag="b1")
                nc.sync.dma_start(out=b1[:], in_=mlpb[0])
                b2 = pool.tile([P, 1], F32, tag="b2")
                nc.sync.dma_start(out=b2[:], in_=mlpb[1])
                b3 = pool.tile([P, 1], F32, tag="b3")
                nc.sync.dma_start(out=b3[:], in_=mlpb[2])

                y1p = pspool.tile([100, G], F32, tag="y1p")
                nc.tensor.matmul(out=y1p[:], lhsT=w1a[:], rhs=mA[:], start=True, stop=False)
                nc.tensor.matmul(out=y1p[:], lhsT=w1b[:], rhs=mB[:], start=False, stop=True)
                y1 = pool.tile([100, G], F32, tag="y1")
                nc.scalar.activation(out=y1[:], in_=y1p[:], func=AF.Relu, bias=b1[0:100, :])
                y2p = pspool.tile([100, G], F32, tag="y2p")
                nc.tensor.matmul(out=y2p[:], lhsT=w2t[:], rhs=y1[:], start=True, stop=True)
                y2 = pool.tile([100, G], F32, tag="y2")
                nc.scalar.activation(out=y2[:], in_=y2p[:], func=AF.Relu, bias=b2[0:100, :])
                y3p = pspool.tile([29, G], F32, tag="y3p")
                nc.tensor.matmul(out=y3p[:], lhsT=w3t[:], rhs=y2[:], start=True, stop=True)
                y3 = pool.tile([29, G], F32, tag="y3")
                nc.scalar.activation(out=y3[:], in_=y3p[:], func=AF.Identity, bias=b3[0:29, :])
                nc.sync.dma_start(out=out[:], in_=y3[:])

    nc.finalize()
    return nc


def make_in_maps(per_core, shared):
    return [{**pc, **shared} for pc in per_core]


# ---------------- runner (device-resident SPMD invoke) ----------------
import jax
from jax.sharding import Mesh, PartitionSpec, NamedSharding
from jax.experimental.shard_map import shard_map
from concourse import bass2jax
from concourse.bass2jax import _bass_exec_p, install_neuronx_cc_hook, partition_id_tensor


class SpmdRunner:
    def __init__(self, nc, n_cores=8):
        install_neuronx_cc_hook()
        self.nc = nc
        self.n_cores = n_cores
        partition_name = nc.partition_id_tensor.name if nc.partition_id_tensor else None
        in_names, out_names, out_avals, zero_outs = [], [], [], []
        for alloc in nc.m.functions[0].allocations:
            if not isinstance(alloc, mybir.MemoryLocationSet):
                continue
            name = alloc.memorylocations[0].name
            if alloc.kind == "ExternalInput":
                if name != partition_name and name != (nc.dbg_addr.name if nc.dbg_addr else None):
                    in_names.append(name)
            elif alloc.kind == "ExternalOutput":
                out_names.append(name)
                shape = tuple(alloc.tensor_shape)
                dtype = mybir.dt.np(alloc.dtype)
                out_avals.append(jax.core.ShapedArray(shape, dtype))
                zero_outs.append(np.zeros(shape, dtype))
        self.in_names, self.out_names = in_names, out_names
        self.out_avals, self.zero_outs = out_avals, zero_outs
        n_params, n_outs = len(in_names), len(out_names)
        self.n_params = n_params
        all_in_names = list(in_names) + list(out_names)
        if nc.dbg_addr is not None:
            all_in_names.append(nc.dbg_addr.name)
        if partition_name is not None:
            all_in_names.append(partition_name)
        self.has_dbg = nc.dbg_addr is not None

        def _body(*args):
            operands = list(args)
            if self.has_dbg:
                operands.append(jax.numpy.zeros((1, 2), jax.numpy.uint32))
            if partition_name is not None:
                operands.append(partition_id_tensor())
            outs = _bass_exec_p.bind(
                *operands,
                out_avals=tuple(out_avals),
                in_names=tuple(all_in_names),
                out_names=tuple(out_names),
                lowering_input_output_aliases=(),
                sim_require_finite=False,
                sim_require_nnan=False,
                nc=nc,
            )
            return tuple(outs)

        devices = jax.devices()[:n_cores]
        self.mesh = Mesh(np.asarray(devices), ("core",))
        in_specs = (PartitionSpec("core"),) * (n_params + n_outs)
        out_specs = (PartitionSpec("core"),) * n_outs
        donate = tuple(range(n_params, n_params + n_outs))
        self.sharded = jax.jit(
            shard_map(_body, mesh=self.mesh, in_specs=in_specs,
                      out_specs=out_specs, check_rep=False),
            donate_argnums=donate, keep_unused=True,
        )
        self.sharding = NamedSharding(self.mesh, PartitionSpec("core"))
        self.dev_in = None

    def stage_inputs(self, in_maps):
        per_core = [[np.asarray(m[n]) for n in self.in_names] for m in in_maps]
        concat_in = [
            np.concatenate([per_core[c][i] for c in range(self.n_cores)], axis=0)
            for i in range(self.n_params)
        ]
        self.dev_in = [jax.device_put(a, self.sharding) for a in concat_in]
        for a in self.dev_in:
            a.block_until_ready()

    def __call__(self):
        concat_zeros = [
            jax.device_put(
                np.zeros((self.n_cores * z.shape[0], *z.shape[1:]), z.dtype),
                self.sharding)
            for z in self.zero_outs
        ]
        out = self.sharded(*self.dev_in, *concat_zeros)
        for o in out:
            o.block_until_ready()
        return out

    def results(self, out):
        return [
            {
                name: np.asarray(out[i]).reshape(self.n_cores, *self.out_avals[i].shape)[c]
                for i, name in enumerate(self.out_names)
            }
            for c in range(self.n_cores)
        ]


# ---------------- entry point ----------------
_CACHE = {}


def _get_runner(SMAX):
    if SMAX not in _CACHE:
        nc = build_nc(SMAX)
        _CACHE[SMAX] = SpmdRunner(nc, NCORE)
    return _CACHE[SMAX]


def kernel(**inputs):
    x = np.asarray(inputs["x"], np.float32)
    edge_index = np.asarray(inputs["edge_index"])
    batch = np.asarray(inputs["batch"])
    Ws = [np.asarray(inputs[f"W{i+1}"], np.float32) for i in range(5)]
    asrcs = [np.asarray(inputs[f"asrc{i+1}"], np.float32) for i in range(5)]
    adsts = [np.asarray(inputs[f"adst{i+1}"], np.float32) for i in range(5)]
    bcs = [np.asarray(inputs[f"bc{i+1}"], np.float32) for i in range(5)]
    lws = [np.asarray(inputs[f"lw{i+1}"], np.float32) for i in range(3)]
    lbs = [np.asarray(inputs[f"lb{i+1}"], np.float32) for i in range(3)]
    per_core, shared, meta = prep(x, edge_index, batch, Ws, asrcs, adsts, bcs, lws, lbs)
    r = _get_runner(meta["SMAX"])
    r.stage_inputs(make_in_maps(per_core, shared))
    out = r()
    y3T = r.results(out)[0]["out"]      # [29, 256] f32
    return np.ascontiguousarray(y3T.T)  # [256, 29]



# revision 12
# speedup vs baseline: 1.6309x; 1.6309x over previous
"""GATNet Trainium kernel v2: host preprocessing + Bass program builder.

Design: 8-way dst-shard of nodes, bf16 everywhere, SBUF-resident activations.
Per layer:
  node phase (per-shard): [h|es|ed] = xT.T @ [W | W@a_s | W@a_d] (PE bf16),
    h|es -> Tshard DRAM (512B rows), ed -> resident SBUF.
  exchange: AllGather Tshard (bf16) -> Tfull.
  aggregation (per-shard dst windows of 128, self-loops handled analytically):
    - J_w indirect-DMA gathers of 128 edge-src rows each (512B rows)
    - one-hot S generated on-device (DVE is_equal vs iota); S^T via
      PE partition-broadcast matmul + is_equal (for dst-score broadcast)
    - w = exp(leakyrelu(es_src + ed_dst)); V = [w*h | w] bf16
    - segment-sum via PE matmuls; self-loop folded in via identity matmul
    - epilogue: out/s + bias, relu -> resident xT via PE transpose
  pool: node->graph one-hot matmuls + AllReduce + tiny MLP.
"""
import sys
sys.path.insert(0, "/opt/trn_rl_repo")
sys.path.insert(0, "/opt/trn_rl_repo/concourse")
import numpy as np
import ml_dtypes
from concourse import bass, bacc, mybir
import concourse.tile as tile

BF16 = mybir.dt.bfloat16
F32 = mybir.dt.float32
I32 = mybir.dt.int32
AF = mybir.ActivationFunctionType
ALU = mybir.AluOpType

N = 50000
E = 400000
G = 256
H = 2
C = 100
HC = 200
F_IN = 336
SLOPE = 0.2
NCORE = 8
SHARD = N // NCORE            # 6250
P = 128
SHARD_PAD = 6272              # 49*128
NW = 49
NT = 49
TROW = 256                    # T table row: [h(200) | es(2) | pad] bf16 = 512B
NFULL_PAD = NCORE * SHARD_PAD
BW = ml_dtypes.bfloat16


def prep(x, edge_index, batch, Ws, asrcs, adsts, bcs, lws, lbs):
    """Host-side preprocessing. Returns (per_core_inputs, shared_inputs, meta)."""
    src_all = edge_index[0].astype(np.int64)
    dst_all = edge_index[1].astype(np.int64)
    order = np.argsort(dst_all, kind="stable")
    s_sorted = src_all[order]
    d_sorted = dst_all[order]

    # window edge ranges + per-window J (shared across cores)
    lo_hi = np.zeros((NCORE, NW, 2), np.int64)
    for k in range(NCORE):
        for w in range(NW):
            d_lo = k * SHARD + w * P
            d_hi = min(k * SHARD + (w + 1) * P, (k + 1) * SHARD)
            lo_hi[k, w, 0] = np.searchsorted(d_sorted, d_lo, side="left")
            lo_hi[k, w, 1] = np.searchsorted(d_sorted, d_hi, side="left")
    cnt = lo_hi[:, :, 1] - lo_hi[:, :, 0]
    J_list = [max(1, int(-(-cnt[:, w].max() // P))) for w in range(NW)]
    SMAX = max(J_list)
    off = np.zeros(NW + 1, np.int64)
    off[1:] = np.cumsum(J_list)
    SJ = int(off[NW])

    per_core = []
    for k in range(NCORE):
        gidx = np.zeros((P, SJ), np.int32)
        mw = np.full((P, SJ), 999.0, np.float32)
        mwT = np.full((NW, SMAX * P), 999.0, np.float32)
        for w in range(NW):
            lo, hi = lo_hi[k, w]
            n = hi - lo
            if n == 0:
                continue
            s = s_sorted[lo:hi]
            srow = ((s // SHARD) * SHARD_PAD + (s % SHARD)).astype(np.int32)
            m = (d_sorted[lo:hi] - (k * SHARD + w * P)).astype(np.float32)
            sl = np.arange(n)
            p_ = sl % P
            j_ = sl // P
            gidx[p_, off[w] + j_] = srow
            mw[p_, off[w] + j_] = m
            mwT[w, j_ * P + p_] = m
        xT0 = np.zeros((384, SHARD_PAD), BW)
        xT0[:F_IN, :SHARD] = x[k * SHARD:(k + 1) * SHARD].T.astype(BW)
        pmask = np.zeros((NT, P, G), BW)
        bsh = batch[k * SHARD:(k + 1) * SHARD]
        for t in range(NT):
            r0 = t * P
            r1 = min(r0 + P, SHARD)
            if r1 > r0:
                pmask[t, np.arange(r1 - r0), bsh[r0:r1]] = 1.0
        per_core.append(dict(xT0=xT0, gidx=gidx, mw=mw.astype(BW), mwT=mwT,
                             pmask=pmask))

    # shared weights
    waug = np.zeros((11, P, 204), BW)
    biasrep = np.zeros((5, P, 202), np.float32)
    ci = 0
    for li in range(5):
        W = Ws[li].astype(np.float32)
        a_s, a_d = asrcs[li].astype(np.float32), adsts[li].astype(np.float32)
        was = np.zeros((W.shape[0], 2), np.float32)
        wad = np.zeros((W.shape[0], 2), np.float32)
        for h in range(H):
            was[:, h] = W[:, h * C:(h + 1) * C] @ a_s[h]
            wad[:, h] = W[:, h * C:(h + 1) * C] @ a_d[h]
        aug = np.concatenate([W, was, wad], axis=1)  # [F, 204]
        nch = 3 if li == 0 else 2
        for c in range(nch):
            rows = aug[c * P:(c + 1) * P]
            waug[ci, :rows.shape[0]] = rows.astype(BW)
            ci += 1
        biasrep[li, :, 0:200] = bcs[li].astype(np.float32)
    assert ci == 11

    mlw1 = lws[0].astype(BW)
    mlw2 = lws[1].astype(BW)
    mlw3 = lws[2].astype(BW)
    mlpb = np.zeros((3, P, 1), np.float32)
    mlpb[0, :100, 0] = lbs[0]
    mlpb[1, :100, 0] = lbs[1]
    mlpb[2, :29, 0] = lbs[2]
    ident = np.eye(P, dtype=BW)
    iota = np.broadcast_to(np.arange(P, dtype=np.float32), (P, P)).astype(BW).copy()
    iotap = np.arange(P, dtype=np.float32).reshape(P, 1).copy()

    shared = dict(waug=waug, biasrep=biasrep, mlw1=mlw1, mlw2=mlw2, mlw3=mlw3,
                  mlpb=mlpb, ident=ident, iota=iota, iotap=iotap)
    meta = dict(SMAX=SMAX, J_list=tuple(J_list), off=off)
    return per_core, shared, meta


def build_nc(SMAX, J_list, off):
    nc = bacc.Bacc("TRN2", target_bir_lowering=False)
    SJ = int(off[NW])

    # inputs
    xT0 = nc.declare_dram_parameter("xT0", [384, SHARD_PAD], BF16, isOutput=False)
    gidx = nc.declare_dram_parameter("gidx", [P, SJ], I32, isOutput=False)
    mw_d = nc.declare_dram_parameter("mw", [P, SJ], BF16, isOutput=False)
    mwT_d = nc.declare_dram_parameter("mwT", [NW, SMAX * P], F32, isOutput=False)
    pmask = nc.declare_dram_parameter("pmask", [NT, P, G], BF16, isOutput=False)
    waug = nc.declare_dram_parameter("waug", [11, P, 204], BF16, isOutput=False)
    biasrep = nc.declare_dram_parameter("biasrep", [5, P, 202], F32, isOutput=False)
    mlw1 = nc.declare_dram_parameter("mlw1", [200, 100], BF16, isOutput=False)
    mlw2 = nc.declare_dram_parameter("mlw2", [100, 100], BF16, isOutput=False)
    mlw3 = nc.declare_dram_parameter("mlw3", [100, 29], BF16, isOutput=False)
    mlpb = nc.declare_dram_parameter("mlpb", [3, P, 1], F32, isOutput=False)
    ident_in = nc.declare_dram_parameter("ident", [P, P], BF16, isOutput=False)
    iota_in = nc.declare_dram_parameter("iota", [P, P], BF16, isOutput=False)
    iotap_in = nc.declare_dram_parameter("iotap", [P, 1], F32, isOutput=False)
    out = nc.declare_dram_parameter("out", [29, G], F32, isOutput=True)

    # internal dram
    Tshard = nc.dram_tensor("Tshard", [SHARD_PAD, TROW], BF16)
    Tfull = nc.dram_tensor("Tfull", [NFULL_PAD, TROW], BF16, addr_space="Shared")
    cc2_in = nc.dram_tensor("cc2_in", [201, G], F32)
    cc2_out = nc.dram_tensor("cc2_out", [201, G], F32, addr_space="Shared")
    rg = [list(range(NCORE))]

    with tile.TileContext(nc) as tc:
        with tc.tile_pool(name="const", bufs=1) as cpool:
            wtiles = []
            for i in range(11):
                wt = cpool.tile([P, 204], BF16, tag=f"waug{i}")
                nc.sync.dma_start(out=wt[:], in_=waug[i])
                wtiles.append(wt)
            ident = cpool.tile([P, P], BF16, tag="ident")
            nc.sync.dma_start(out=ident[:], in_=ident_in[:])
            iota_sb = cpool.tile([P, P], BF16, tag="iota")
            nc.sync.dma_start(out=iota_sb[:], in_=iota_in[:])
            iotap = cpool.tile([P, 1], F32, tag="iotap")
            nc.sync.dma_start(out=iotap[:], in_=iotap_in[:])
            ones1f = cpool.tile([1, P], F32, tag="ones1f")
            nc.vector.memset(ones1f[:], 1.0)
            gidx_sb = cpool.tile([P, SJ], I32, tag="gidx")
            nc.sync.dma_start(out=gidx_sb[:], in_=gidx[:])
            mw_sb = cpool.tile([P, SJ], BF16, tag="mw")
            nc.sync.dma_start(out=mw_sb[:], in_=mw_d[:])
            xTa = cpool.tile([P, SHARD_PAD], BF16, tag="xTa")
            nc.vector.memset(xTa[:], 0.0)
            xTb = cpool.tile([P, SHARD_PAD], BF16, tag="xTb")
            nc.vector.memset(xTb[:], 0.0)
            x6_sb = cpool.tile([P, NT, 204], BF16, tag="x6")
            nc.vector.memset(x6_sb[:], 0.0)
            nc.vector.memset(x6_sb[:, :, 200:201], 1.0)
            zpad = cpool.tile([SHARD_PAD - SHARD, TROW], BF16, tag="zpad")
            nc.vector.memset(zpad[:], 0.0)
            nc.sync.dma_start(out=Tshard[SHARD:SHARD_PAD, :], in_=zpad[:])

            with tc.tile_pool(name="sb", bufs=2) as pool, \
                 tc.tile_pool(name="sb3", bufs=3) as pool3:
              with tc.tile_pool(name="ps", bufs=2, space="PSUM") as pspool, \
                   tc.tile_pool(name="psb", bufs=1, space="PSUM") as pspoolb:

                def node_phase(li, nch, ci0):
                    for t in range(NT):
                        r0 = t * P if t < NT - 1 else SHARD - P
                        npsum = pspool.tile([P, 204], F32, tag="npsum")
                        for c in range(nch):
                            if li == 0:
                                lt = pool3.tile([P, P], BF16, tag="nlhsT")
                                nc.sync.dma_start(
                                    out=lt[:],
                                    in_=xT0[c * P:(c + 1) * P, r0:r0 + P])
                                lhsT = lt[:]
                                rhs = wtiles[ci0 + c][:]
                            else:
                                if c == 0:
                                    lhsT = xTa[:, r0:r0 + P]
                                    rhs = wtiles[ci0 + c][:]
                                else:
                                    lhsT = xTb[0:72, r0:r0 + P]
                                    rhs = wtiles[ci0 + c][0:72, :]
                            nc.tensor.matmul(
                                out=npsum[:], lhsT=lhsT, rhs=rhs,
                                start=(c == 0), stop=(c == nch - 1))
                        ts = pool.tile([P, 204], BF16, tag="tstage")
                        nc.scalar.activation(out=ts[:], in_=npsum[:], func=AF.Copy)
                        nc.sync.dma_start(out=Tshard[r0:r0 + P, 0:204], in_=ts[:])

                def agg_phase(li):
                    last = li == 4
                    brt = pool.tile([P, 202], F32, tag="biasrep")
                    nc.sync.dma_start(out=brt[:], in_=biasrep[li])
                    for w in range(NW):
                        J = J_list[w]
                        o0 = int(off[w])
                        Gt = pool.tile([P, SMAX, TROW], BF16, tag="Gt")
                        for j in range(J):
                            nc.gpsimd.indirect_dma_start(
                                out=Gt[:, j, :], out_offset=None, in_=Tfull[:],
                                in_offset=bass.IndirectOffsetOnAxis(
                                    ap=gidx_sb[:, o0 + j:o0 + j + 1], axis=0))
                        # one-hot S [slot(p,j), dst m]
                        st = pool.tile([P, SMAX, P], BF16, tag="st")
                        nc.vector.tensor_tensor(
                            out=st[:, 0:J, :],
                            in0=iota_sb[:].unsqueeze(1).broadcast_to([P, J, P]),
                            in1=mw_sb[:, o0:o0 + J].unsqueeze(2).broadcast_to([P, J, P]),
                            op=ALU.is_equal)
                        # S^T [dst m, slot(j,p)] via partition-broadcast matmul
                        mwT_t = pool.tile([1, SMAX * P], F32, tag="mwT")
                        nc.sync.dma_start(out=mwT_t[:, 0:J * P], in_=mwT_d[w, 0:J * P])
                        stt = pool.tile([P, SMAX * P], BF16, tag="stt")
                        for j in range(J):
                            bp = pspoolb.tile([P, P], F32, tag="bp")
                            nc.tensor.matmul(
                                out=bp[:], lhsT=ones1f[:],
                                rhs=mwT_t[0:1, j * P:(j + 1) * P],
                                start=True, stop=True)
                            nc.vector.tensor_scalar(
                                out=stt[:, j * P:(j + 1) * P], in0=bp[:],
                                scalar1=iotap[:, 0:1], scalar2=None,
                                op0=ALU.is_equal)
                        # window's own dst rows (self-loop data + dst scores)
                        hT = pool.tile([P, 204], BF16, tag="hT")
                        nc.sync.dma_start(out=hT[:], in_=Tshard[w * P:(w + 1) * P, 0:204])
                        # dst scores per slot
                        edp = pspoolb.tile([P, 2 * SMAX], F32, tag="edp")
                        for j in range(J):
                            nc.tensor.matmul(
                                out=edp[:, 2 * j:2 * j + 2],
                                lhsT=stt[:, j * P:(j + 1) * P],
                                rhs=hT[:, 202:204], start=True, stop=True)
                        # w = exp(leakyrelu(es+ed))
                        e1 = pool.tile([P, SMAX, 2], F32, tag="e1")
                        nc.vector.tensor_add(
                            out=e1[:, 0:J, :], in0=Gt[:, 0:J, 200:202],
                            in1=edp[:].rearrange("p (j c) -> p j c", c=2)[:, 0:J, :])
                        e2 = pool.tile([P, SMAX, 2], F32, tag="e2")
                        nc.vector.tensor_scalar_mul(out=e2[:, 0:J, :], in0=e1[:, 0:J, :],
                                                    scalar1=SLOPE)
                        e3 = pool.tile([P, SMAX, 2], F32, tag="e3")
                        nc.vector.tensor_tensor(out=e3[:, 0:J, :], in0=e1[:, 0:J, :],
                                                in1=e2[:, 0:J, :], op=ALU.max)
                        wv = pool.tile([P, SMAX, 2], BF16, tag="wv")
                        nc.scalar.activation(out=wv[:, 0:J, :], in_=e3[:, 0:J, :],
                                             func=AF.Exp)
                        # V = [w*h | w]
                        Vt = pool.tile([P, SMAX, 202], BF16, tag="Vt")
                        nc.vector.tensor_mul(
                            out=Vt[:, 0:J, 0:200].rearrange("p j (h c) -> p j h c", h=H),
                            in0=Gt[:, 0:J, 0:200].rearrange("p j (h c) -> p j h c", h=H),
                            in1=wv[:, 0:J, :].unsqueeze(3).broadcast_to([P, J, 2, C]))
                        nc.vector.tensor_copy(out=Vt[:, 0:J, 200:202], in_=wv[:, 0:J, :])
                        # self-loop V
                        e1s = pool.tile([P, 2], F32, tag="e1s")
                        nc.vector.tensor_add(out=e1s[:], in0=hT[:, 200:202],
                                             in1=hT[:, 202:204])
                        e2s = pool.tile([P, 2], F32, tag="e2s")
                        nc.vector.tensor_scalar_mul(out=e2s[:], in0=e1s[:], scalar1=SLOPE)
                        e3s = pool.tile([P, 2], F32, tag="e3s")
                        nc.vector.tensor_tensor(out=e3s[:], in0=e1s[:], in1=e2s[:],
                                                op=ALU.max)
                        ws = pool.tile([P, 2], BF16, tag="ws")
                        nc.scalar.activation(out=ws[:], in_=e3s[:], func=AF.Exp)
                        Vs = pool.tile([P, 202], BF16, tag="Vs")
                        nc.vector.tensor_mul(
                            out=Vs[:, 0:200].rearrange("p (h c) -> p h c", h=H),
                            in0=hT[:, 0:200].rearrange("p (h c) -> p h c", h=H),
                            in1=ws[:].unsqueeze(2).broadcast_to([P, 2, C]))
                        nc.vector.tensor_copy(out=Vs[:, 200:202], in_=ws[:])
                        # segment sum (self-loop via identity matmul)
                        apsum = pspool.tile([P, 202], F32, tag="apsum")
                        nc.tensor.matmul(out=apsum[:], lhsT=ident[:], rhs=Vs[:],
                                         start=True, stop=False)
                        for j in range(J):
                            nc.tensor.matmul(
                                out=apsum[:], lhsT=st[:, j, :], rhs=Vt[:, j, :],
                                start=False, stop=(j == J - 1))
                        # epilogue
                        rc = pool.tile([P, 2], F32, tag="rc")
                        nc.vector.reciprocal(out=rc[:], in_=apsum[:, 200:202])
                        stg = pool.tile([P, 202], F32, tag="stg")
                        nc.vector.tensor_scalar_mul(out=stg[:, 0:100],
                                                    in0=apsum[:, 0:100],
                                                    scalar1=rc[:, 0:1])
                        nc.vector.tensor_scalar_mul(out=stg[:, 100:202],
                                                    in0=apsum[:, 100:202],
                                                    scalar1=rc[:, 1:2])
                        nc.vector.tensor_add(out=stg[:], in0=stg[:], in1=brt[:])
                        stb = pool.tile([P, 202], BF16, tag="stb")
                        nc.vector.tensor_scalar_max(out=stb[:], in0=stg[:], scalar1=0.0)
                        if not last:
                            tp1 = pspoolb.tile([P, P], BF16, tag="tp1")
                            nc.tensor.transpose(out=tp1[:], in_=stb[:, 0:128],
                                                identity=ident[:])
                            tp2 = pspoolb.tile([P, P], BF16, tag="tp2")
                            nc.tensor.transpose(out=tp2[0:72, :], in_=stb[:, 128:200],
                                                identity=ident[:])
                            c0 = w * P
                            nc.vector.tensor_copy(out=xTa[:, c0:c0 + P], in_=tp1[:])
                            nc.vector.tensor_copy(out=xTb[0:72, c0:c0 + P],
                                                  in_=tp2[0:72, :])
                        else:
                            nc.vector.tensor_copy(out=x6_sb[:, w, 0:200],
                                                  in_=stb[:, 0:200])

                # ---- 5 GAT layers ----
                for li in range(5):
                    nch = 3 if li == 0 else 2
                    ci0 = 0 if li == 0 else 3 + 2 * (li - 1)
                    node_phase(li, nch, ci0)
                    tc.strict_bb_all_engine_barrier()
                    nc.gpsimd.collective_compute(
                        "AllGather", ALU.bypass, replica_groups=rg,
                        ins=[Tshard[:]], outs=[Tfull[:]])
                    tc.strict_bb_all_engine_barrier()
                    agg_phase(li)
                    tc.strict_bb_all_engine_barrier()

              # ---- pool ----
              with tc.tile_pool(name="ps2", bufs=1, space="PSUM") as pspool:
                ppA = pspool.tile([P, G], F32, tag="ppA")
                ppB = pspool.tile([73, G], F32, tag="ppB")
                for t in range(NT):
                    pmt = pool3.tile([P, G], BF16, tag="pmt")
                    nc.sync.dma_start(out=pmt[:], in_=pmask[t])
                    nc.tensor.matmul(out=ppA[:], lhsT=x6_sb[:, t, 0:128], rhs=pmt[:],
                                     start=(t == 0), stop=(t == NT - 1))
                    nc.tensor.matmul(out=ppB[:], lhsT=x6_sb[:, t, 128:201], rhs=pmt[:],
                                     start=(t == 0), stop=(t == NT - 1))
                pstA = pool.tile([P, G], F32, tag="pstA")
                nc.vector.tensor_copy(out=pstA[:], in_=ppA[:])
                pstB = pool.tile([73, G], F32, tag="pstB")
                nc.vector.tensor_copy(out=pstB[:], in_=ppB[:])
                nc.sync.dma_start(out=cc2_in[0:128, :], in_=pstA[:])
                nc.sync.dma_start(out=cc2_in[128:201, :], in_=pstB[:])
                tc.strict_bb_all_engine_barrier()
                nc.gpsimd.collective_compute(
                    "AllReduce", ALU.add, replica_groups=rg,
                    ins=[cc2_in[:]], outs=[cc2_out[:]])
                tc.strict_bb_all_engine_barrier()
                plA = pool.tile([P, G], F32, tag="plA")
                nc.sync.dma_start(out=plA[:], in_=cc2_out[0:128, :])
                plB = pool.tile([73, G], F32, tag="plB")
                nc.sync.dma_start(out=plB[:], in_=cc2_out[128:201, :])
                cntr = pool.tile([1, G], F32, tag="cntr")
                nc.sync.dma_start(out=cntr[:], in_=cc2_out[200:201, :])
                cntm = pool.tile([1, G], F32, tag="cntm")
                nc.vector.tensor_scalar_max(out=cntm[:], in0=cntr[:], scalar1=1.0)
                rc2 = pool.tile([1, G], F32, tag="rc2")
                nc.vector.reciprocal(out=rc2[:], in_=cntm[:])
                Rb = pspool.tile([P, G], F32, tag="Rb")
                nc.tensor.matmul(out=Rb[:], lhsT=ones1f[:], rhs=rc2[:],
                                 start=True, stop=True)
                mA = pool.tile([P, G], BF16, tag="mA")
                nc.vector.tensor_mul(out=mA[:], in0=plA[:], in1=Rb[:])
                mB = pool.tile([72, G], BF16, tag="mB")
                nc.vector.tensor_mul(out=mB[:], in0=plB[0:72, :], in1=Rb[0:72, :])
                # MLP
                w1a = pool.tile([P, 100], BF16, tag="w1a")
                nc.sync.dma_start(out=w1a[:], in_=mlw1[0:128, :])
                w1b = pool.tile([72, 100], BF16, tag="w1b")
                nc.sync.dma_start(out=w1b[:], in_=mlw1[128:200, :])
                w2t = pool.tile([100, 100], BF16, tag="w2t")
                nc.sync.dma_start(out=w2t[:], in_=mlw2[:])
                w3t = pool.tile([100, 29], BF16, tag="w3t")
                nc.sync.dma_start(out=w3t[:], in_=mlw3[:])
                b1 = pool.tile([P, 1], F32, tag="b1")
                nc.sync.dma_start(out=b1[:], in_=mlpb[0])
                b2 = pool.tile([P, 1], F32, tag="b2")
                nc.sync.dma_start(out=b2[:], in_=mlpb[1])
                b3 = pool.tile([P, 1], F32, tag="b3")
                nc.sync.dma_start(out=b3[:], in_=mlpb[2])

                y1p = pspool.tile([100, G], F32, tag="y1p")
                nc.tensor.matmul(out=y1p[:], lhsT=w1a[:], rhs=mA[:], start=True, stop=False)
                nc.tensor.matmul(out=y1p[:], lhsT=w1b[:], rhs=mB[:], start=False, stop=True)
                y1 = pool.tile([100, G], BF16, tag="y1")
                nc.scalar.activation(out=y1[:], in_=y1p[:], func=AF.Relu, bias=b1[0:100, :])
                y2p = pspool.tile([100, G], F32, tag="y2p")
                nc.tensor.matmul(out=y2p[:], lhsT=w2t[:], rhs=y1[:], start=True, stop=True)
                y2 = pool.tile([100, G], BF16, tag="y2")
                nc.scalar.activation(out=y2[:], in_=y2p[:], func=AF.Relu, bias=b2[0:100, :])
                y3p = pspool.tile([29, G], F32, tag="y3p")
                nc.tensor.matmul(out=y3p[:], lhsT=w3t[:], rhs=y2[:], start=True, stop=True)
                y3 = pool.tile([29, G], F32, tag="y3")
                nc.scalar.activation(out=y3[:], in_=y3p[:], func=AF.Identity, bias=b3[0:29, :])
                nc.sync.dma_start(out=out[:], in_=y3[:])

    nc.finalize()
    return nc


def make_in_maps(per_core, shared):
    return [{**pc, **shared} for pc in per_core]


# ---------------- runner (device-resident SPMD invoke) ----------------
import jax
from jax.sharding import Mesh, PartitionSpec, NamedSharding
from jax.experimental.shard_map import shard_map
from concourse import bass2jax
from concourse.bass2jax import _bass_exec_p, install_neuronx_cc_hook, partition_id_tensor


class SpmdRunner:
    def __init__(self, nc, n_cores=8):
        install_neuronx_cc_hook()
        self.nc = nc
        self.n_cores = n_cores
        partition_name = nc.partition_id_tensor.name if nc.partition_id_tensor else None
        in_names, out_names, out_avals, zero_outs = [], [], [], []
        for alloc in nc.m.functions[0].allocations:
            if not isinstance(alloc, mybir.MemoryLocationSet):
                continue
            name = alloc.memorylocations[0].name
            if alloc.kind == "ExternalInput":
                if name != partition_name and name != (nc.dbg_addr.name if nc.dbg_addr else None):
                    in_names.append(name)
            elif alloc.kind == "ExternalOutput":
                out_names.append(name)
                shape = tuple(alloc.tensor_shape)
                dtype = mybir.dt.np(alloc.dtype)
                out_avals.append(jax.core.ShapedArray(shape, dtype))
                zero_outs.append(np.zeros(shape, dtype))
        self.in_names, self.out_names = in_names, out_names
        self.out_avals, self.zero_outs = out_avals, zero_outs
        n_params, n_outs = len(in_names), len(out_names)
        self.n_params = n_params
        all_in_names = list(in_names) + list(out_names)
        if nc.dbg_addr is not None:
            all_in_names.append(nc.dbg_addr.name)
        if partition_name is not None:
            all_in_names.append(partition_name)
        self.has_dbg = nc.dbg_addr is not None

        def _body(*args):
            operands = list(args)
            if self.has_dbg:
                operands.append(jax.numpy.zeros((1, 2), jax.numpy.uint32))
            if partition_name is not None:
                operands.append(partition_id_tensor())
            outs = _bass_exec_p.bind(
                *operands,
                out_avals=tuple(out_avals),
                in_names=tuple(all_in_names),
                out_names=tuple(out_names),
                lowering_input_output_aliases=(),
                sim_require_finite=False,
                sim_require_nnan=False,
                nc=nc,
            )
            return tuple(outs)

        devices = jax.devices()[:n_cores]
        self.mesh = Mesh(np.asarray(devices), ("core",))
        in_specs = (PartitionSpec("core"),) * (n_params + n_outs)
        out_specs = (PartitionSpec("core"),) * n_outs
        donate = tuple(range(n_params, n_params + n_outs))
        self.sharded = jax.jit(
            shard_map(_body, mesh=self.mesh, in_specs=in_specs,
                      out_specs=out_specs, check_rep=False),
            donate_argnums=donate, keep_unused=True,
        )
        self.sharding = NamedSharding(self.mesh, PartitionSpec("core"))
        self.dev_in = None

    def stage_inputs(self, in_maps):
        per_core = [[np.asarray(m[n]) for n in self.in_names] for m in in_maps]
        concat_in = [
            np.concatenate([per_core[c][i] for c in range(self.n_cores)], axis=0)
            for i in range(self.n_params)
        ]
        self.dev_in = [jax.device_put(a, self.sharding) for a in concat_in]
        for a in self.dev_in:
            a.block_until_ready()

    def __call__(self):
        concat_zeros = [
            jax.device_put(
                np.zeros((self.n_cores * z.shape[0], *z.shape[1:]), z.dtype),
                self.sharding)
            for z in self.zero_outs
        ]
        out = self.sharded(*self.dev_in, *concat_zeros)
        for o in out:
            o.block_until_ready()
        return out

    def results(self, out):
        return [
            {
                name: np.asarray(out[i]).reshape(self.n_cores, *self.out_avals[i].shape)[c]
                for i, name in enumerate(self.out_names)
            }
            for c in range(self.n_cores)
        ]


# ---------------- entry point ----------------
_CACHE = {}


def _get_runner(meta):
    key = (meta["SMAX"], meta["J_list"])
    if key not in _CACHE:
        nc = build_nc(meta["SMAX"], list(meta["J_list"]), meta["off"])
        _CACHE[key] = SpmdRunner(nc, NCORE)
    return _CACHE[key]


def kernel(**inputs):
    x = np.asarray(inputs["x"], np.float32)
    edge_index = np.asarray(inputs["edge_index"])
    batch = np.asarray(inputs["batch"])
    Ws = [np.asarray(inputs[f"W{i+1}"], np.float32) for i in range(5)]
    asrcs = [np.asarray(inputs[f"asrc{i+1}"], np.float32) for i in range(5)]
    adsts = [np.asarray(inputs[f"adst{i+1}"], np.float32) for i in range(5)]
    bcs = [np.asarray(inputs[f"bc{i+1}"], np.float32) for i in range(5)]
    lws = [np.asarray(inputs[f"lw{i+1}"], np.float32) for i in range(3)]
    lbs = [np.asarray(inputs[f"lb{i+1}"], np.float32) for i in range(3)]
    per_core, shared, meta = prep(x, edge_index, batch, Ws, asrcs, adsts, bcs, lws, lbs)
    r = _get_runner(meta)
    r.stage_inputs(make_in_maps(per_core, shared))
    out = r()
    y3T = r.results(out)[0]["out"]      # [29, 256] f32
    return np.ascontiguousarray(y3T.T)  # [256, 29]
